# revision 35
# baseline (speedup 1.0000x reference)
"""Trainium2 Bass kernel for Bahdanau-style attention scoring (sparse_attention).

Math (per reference):
    u1 = W[:, :H].T @ v ; u2 = W[:, H:].T @ v ; c = b @ v
    sh[b, n] = hidden[n, b, :] @ u1
    se[b, t] = encoder_outputs[t, b, :] @ u2
    out[b, n, t] = softmax_t(tanh(sh[b, n] + se[b, t] + c))

Sharding: data-parallel over batch B=64 across 8 cores (8 batch rows per
core); the small attn weights are replicated to every core in their
reference-decomposed form (u1/u2/c, the same decomposition reference.py
itself uses). No collectives.

v6 design:
  - All device I/O bf16 (host converts/relayouts; rel_err ~6e-3 vs 2e-2
    gate). ~9.1MB/core vs 18.9MB fp32.
  - enc/hid shipped pre-transposed so TensorE does all dot products:
      pre[n,t] = se[t]: lhsT = u2bc (u2bc[h,n] = u2[h]), rhs = encT.
      shc[n,j]: lhsT = hidT chunk, rhs = u1 column; +c on the
      PSUM->SBUF Vector copy.
  - ScalarE critical path per b: 2 bias'd tanh (the bias is per-
    partition so the two n-chunks can't merge) + ONE 2048-wide merged
    exp; sums via a single VectorE tensor_reduce per b.
  - encT laid out [b, hp, th, hc, t'] so b0's load splits into two
    contiguous halves and ScalarE starts on the first half early.
  - Last batch: unmerged exps with ACT accum + per-j scale/store on the
    idle ACT ring for a short tail.
"""

import os
import sys

import numpy as np

for _p in ("/opt/trn_rl_repo", "/root/.axon_site/_ro/trn_rl_repo"):
    if os.path.isdir(_p) and _p not in sys.path:
        sys.path.insert(0, _p)

from contextlib import ExitStack

import ml_dtypes

import concourse.bass as bass
import concourse.tile as tile
from concourse import bacc, mybir
from concourse.bass_utils import run_bass_kernel_spmd

H = 256
N_LEN = 256
T_LEN = 1024
BATCH = 64
NCORES = 8
B_LOC = BATCH // NCORES  # 8
P = 128
FP32 = mybir.dt.float32
BF16 = mybir.dt.bfloat16
AF = mybir.ActivationFunctionType
ALU = mybir.AluOpType
BF16_NP = ml_dtypes.bfloat16


def build_program():
    nc = bacc.Bacc(
        "TRN2",
        target_bir_lowering=False,
        debug=False,
        enable_asserts=True,
        num_devices=NCORES,
    )

    # Host-prepared layouts (see make_in_maps):
    #   encT[b, hp, th, hc, t'] = enc[th*512+t', b, hc*128+hp]   bf16
    #   hidT[b, hp, hc, n] = hid[n, b, hc*128+hp]                bf16
    enc_ap = nc.dram_tensor(
        "encT", [B_LOC, P, 2, 2, 512], BF16, kind="ExternalInput"
    ).ap()
    hid_ap = nc.dram_tensor("hidT", [B_LOC, P, 2, N_LEN], BF16, kind="ExternalInput").ap()
    # wpack[:, hc, 0:128]=u2bc, [:, hc, 128]=u1 col hc,
    # [:, 0, 130:132]=fp32 bits of c (bitcast on device)
    wpack_ap = nc.dram_tensor("wpack", [P, 2, 132], BF16, kind="ExternalInput").ap()
    out_ap = nc.dram_tensor(
        "out", [B_LOC, 2, P, T_LEN], BF16, kind="ExternalOutput"
    ).ap()

    out_r = out_ap.rearrange("b nc p t -> b p nc t")  # (8, 128, 2, 1024)

    with tile.TileContext(nc) as tc, ExitStack() as ctx:
        singles = ctx.enter_context(tc.tile_pool(name="singles", bufs=1))
        ps_pre = ctx.enter_context(tc.tile_pool(name="ps_pre", bufs=3, space="PSUM"))
        ps_shc = ctx.enter_context(tc.tile_pool(name="ps_shc", bufs=2, space="PSUM"))
        enc_pool = ctx.enter_context(tc.tile_pool(name="enc", bufs=8))
        hid_pool = ctx.enter_context(tc.tile_pool(name="hid", bufs=8))
        stats = ctx.enter_context(tc.tile_pool(name="stats", bufs=8))
        et_pool = ctx.enter_context(tc.tile_pool(name="et", bufs=3))
        xt_pool = ctx.enter_context(tc.tile_pool(name="xt", bufs=4))
        ot_pool = ctx.enter_context(tc.tile_pool(name="ot", bufs=3))

        # ---- b0's enc halves + hid0 lead the sync ring; wpack rides the
        #      otherwise-idle ACT ring in parallel ----
        enc_sbs, hid_sbs = [], []
        enc0a = enc_pool.tile([P, 2, 512], BF16, tag="enc0a")
        nc.sync.dma_start(enc0a[:], enc_ap[0][:, 0])
        wpack = singles.tile([P, 2, 132], BF16)
        nc.sync.dma_start(wpack[:], wpack_ap)
        u2bc_sb = wpack[:, :, 0:P]
        u1sb = wpack[:, :, P : P + 1]  # (128, 2, 1)
        c_col = wpack[:, 0, 130:132].bitcast(FP32)  # (128, 1) fp32

        for b in range(B_LOC):
            hid_sb = hid_pool.tile([P, 2, N_LEN], BF16)
            if b == 0:
                # b0: enc in two SEPARATE tiles (the dep tracker is
                # whole-tile; split DMAs into one tile create false
                # cross-half dependencies)
                nc.sync.dma_start(hid_sb[:], hid_ap[b])
                enc0b = enc_pool.tile([P, 2, 512], BF16, tag="enc0b")
                nc.sync.dma_start(enc0b[:], enc_ap[b][:, 1])
                enc_sb = (enc0a, enc0b)
            else:
                enc_sb = enc_pool.tile([P, 2, 2, 512], BF16)
                nc.sync.dma_start(enc_sb[:], enc_ap[b])
                nc.sync.dma_start(hid_sb[:], hid_ap[b])
            enc_sbs.append(enc_sb)
            hid_sbs.append(hid_sb)

        # warm the ACT spline tables off the critical path (tanh and exp
        # live in the same table set; one activation loads it)
        warm_in = singles.tile([1, P], BF16)
        nc.vector.memset(warm_in[:], 1.0)
        warm = singles.tile([1, P], FP32)
        nc.scalar.activation(out=warm[:], in_=warm_in[:], func=AF.Tanh)

        # ---- main pipeline ----
        SKEW = 1
        xts, sums_t = {}, {}

        def normalize_and_store(b):
            rsums = stats.tile([P, 2], FP32, tag="rsums")
            nc.vector.reciprocal(rsums[:], sums_t[b][:])
            o_t = ot_pool.tile([P, 2, T_LEN], BF16)
            for j in range(2):
                nc.vector.tensor_scalar_mul(
                    o_t[:, j, :], xts[(b, j)][:], rsums[:, j : j + 1]
                )
            nc.sync.dma_start(out_r[b], o_t[:])

        for b in range(B_LOC):
            enc_sb, hid_sb = enc_sbs[b], hid_sbs[b]

            # shc[:, j] = hid[n, :] . u1 (PE columns); +c on the copy out
            shc_ps = ps_shc.tile([P, 2], FP32)
            for j in range(2):
                for hc in range(2):
                    nc.tensor.matmul(
                        out=shc_ps[:, j : j + 1],
                        lhsT=hid_sb[:, hc, j * P : (j + 1) * P],
                        rhs=u1sb[:, hc, :],
                        start=(hc == 0),
                        stop=(hc == 1),
                    )
            shc = stats.tile([P, 2], FP32, tag="shc")
            nc.vector.tensor_scalar_add(shc[:], shc_ps[:], c_col)

            # pre[n, t] = se[t] for every n-partition (PE broadcast-dot)
            pre_ps = ps_pre.tile([P, T_LEN], FP32)
            for th in range(2):
                rhs_th = enc_sb[th] if b == 0 else enc_sb[:, th]
                for hc in range(2):
                    nc.tensor.matmul(
                        out=pre_ps[:, th * 512 : (th + 1) * 512],
                        lhsT=u2bc_sb[:, hc, :],
                        rhs=rhs_th[:, hc, :],
                        start=(hc == 0),
                        stop=(hc == 1),
                    )

            # transcendentals: 2 bias'd tanh + 1 merged exp per b
            sums = stats.tile([P, 2], FP32, tag="sums")
            sums_t[b] = sums
            last = b == B_LOC - 1
            e2 = et_pool.tile([P, 2, T_LEN], FP32)
            for j in range(2):
                if b == 0 and j == 0:
                    # split so ScalarE starts on enc0's first half early
                    for th in range(2):
                        nc.scalar.activation(
                            out=e2[:, j, th * 512 : (th + 1) * 512],
                            in_=pre_ps[:, th * 512 : (th + 1) * 512],
                            func=AF.Tanh,
                            bias=shc[:, j : j + 1],
                            scale=1.0,
                        )
                else:
                    nc.scalar.activation(
                        out=e2[:, j, :],
                        in_=pre_ps[:],
                        func=AF.Tanh,
                        bias=shc[:, j : j + 1],
                        scale=1.0,
                    )
            x2 = xt_pool.tile([P, 2, T_LEN], BF16)
            if last:
                # tail: per-j exps with accum so normalize starts immediately
                for j in range(2):
                    xl = xt_pool.tile([P, T_LEN], BF16, tag=f"xl{j}")
                    nc.scalar.activation(
                        out=xl[:],
                        in_=e2[:, j, :],
                        func=AF.Exp,
                        accum_out=sums[:, j : j + 1],
                    )
                    xts[(b, j)] = xl
            else:
                nc.scalar.activation(out=x2[:], in_=e2[:], func=AF.Exp)
                nc.vector.tensor_reduce(
                    out=sums[:], in_=x2[:], axis=mybir.AxisListType.X, op=ALU.add
                )
                xts[(b, 0)] = x2[:, 0, :]
                xts[(b, 1)] = x2[:, 1, :]

            if b >= SKEW:
                normalize_and_store(b - SKEW)

        # tail: per-j scale+store for the final batch; stores ride the
        # now-idle ACT ring
        bl = B_LOC - 1
        for j in range(2):
            rs = stats.tile([P, 1], FP32, tag=f"rs{j}")
            nc.vector.reciprocal(rs[:], sums_t[bl][:, j : j + 1])
            o_j = ot_pool.tile([P, T_LEN], BF16, tag=f"oj{j}")
            nc.vector.tensor_scalar_mul(o_j[:], xts[(bl, j)][:], rs[:])
            nc.scalar.dma_start(out_r[bl][:, j, :], o_j[:])

    nc.compile()
    return nc


_CACHE = {}


def get_program():
    if "nc" not in _CACHE:
        _CACHE["nc"] = build_program()
    return _CACHE["nc"]


def make_in_maps(hidden, encoder_outputs, W, b, v):
    # encT[b, hp, th, hc, t'] = enc[th*512+t', b, hc*128+hp]
    encT = np.asarray(encoder_outputs, dtype=np.float32).reshape(2, 512, BATCH, 2, P)
    encT = encT.transpose(2, 4, 0, 3, 1).astype(BF16_NP)  # (64, 128, 2, 2, 512)
    hidT = np.asarray(hidden, dtype=np.float32).reshape(N_LEN, BATCH, 2, P)
    hidT = hidT.transpose(1, 3, 2, 0).astype(BF16_NP)  # (64, 128, 2, 256)

    # replicated small weights, in the reference's own u1/u2/c decomposition
    W32 = np.asarray(W, dtype=np.float32)
    v32 = np.asarray(v, dtype=np.float32)
    b32 = np.asarray(b, dtype=np.float32)
    u1 = W32[:, :H].T @ v32  # (256,)
    u2 = W32[:, H:].T @ v32  # (256,)
    c = float(b32 @ v32)
    # wpack[:, hc, 0:128]=u2bc, [:, hc, 128]=u1 col hc, [:, 0, 130:132]=c bits
    wpack = np.zeros((P, 2, 132), dtype=np.float32)
    wpack[:, :, 0:P] = u2.reshape(2, P).T[:, :, None]
    wpack[:, :, P] = u1.reshape(2, P).T
    wpack = wpack.astype(BF16_NP)
    c_u16 = np.frombuffer(np.float32(c).tobytes(), dtype=np.uint16)
    wp_u16 = wpack.view(np.uint16)
    wp_u16[:, 0, 130] = c_u16[0]
    wp_u16[:, 0, 131] = c_u16[1]

    in_maps = []
    for i in range(NCORES):
        sl = slice(i * B_LOC, (i + 1) * B_LOC)
        in_maps.append(
            {
                "encT": np.ascontiguousarray(encT[sl]),
                "hidT": np.ascontiguousarray(hidT[sl]),
                "wpack": wpack,
            }
        )
    return in_maps


def kernel(hidden, encoder_outputs, W, b, v, _trace=False, _trace_kwargs=None):
    nc = get_program()
    in_maps = make_in_maps(hidden, encoder_outputs, W, b, v)
    res = run_bass_kernel_spmd(
        nc,
        in_maps,
        core_ids=list(range(NCORES)),
        trace=_trace,
        **(_trace_kwargs or {}),
    )
    parts = []
    for i in range(NCORES):
        o = np.asarray(res.results[i]["out"])  # (8, 2, 128, 1024) bf16
        parts.append(o.reshape(B_LOC, N_LEN, T_LEN).astype(np.float32))
    out = np.concatenate(parts, axis=0)
    if _trace:
        return out, res
    return out


# revision 36
# speedup vs baseline: 1.0400x; 1.0400x over previous
"""Trainium2 Bass kernel for Bahdanau-style attention scoring (sparse_attention).

Math (per reference):
    u1 = W[:, :H].T @ v ; u2 = W[:, H:].T @ v ; c = b @ v
    sh[b, n] = hidden[n, b, :] @ u1
    se[b, t] = encoder_outputs[t, b, :] @ u2
    out[b, n, t] = softmax_t(tanh(sh[b, n] + se[b, t] + c))

Sharding: data-parallel over batch B=64 across 8 cores (8 batch rows per
core); the small attn weights are replicated to every core in their
reference-decomposed form (u1/u2/c, the same decomposition reference.py
itself uses). No collectives.

v6 design:
  - All device I/O bf16 (host converts/relayouts; rel_err ~6e-3 vs 2e-2
    gate). ~9.1MB/core vs 18.9MB fp32.
  - enc/hid shipped pre-transposed so TensorE does all dot products:
      pre[n,t] = se[t]: lhsT = u2bc (u2bc[h,n] = u2[h]), rhs = encT.
      shc[n,j]: lhsT = hidT chunk, rhs = u1 column; +c on the
      PSUM->SBUF Vector copy.
  - ScalarE critical path per b: 2 bias'd tanh (the bias is per-
    partition so the two n-chunks can't merge) + ONE 2048-wide merged
    exp; sums via a single VectorE tensor_reduce per b.
  - encT laid out [b, hp, th, hc, t'] so b0's load splits into two
    contiguous halves and ScalarE starts on the first half early.
  - Last batch: unmerged exps with ACT accum + per-j scale/store on the
    idle ACT ring for a short tail.
"""

import os
import sys

import numpy as np

for _p in ("/opt/trn_rl_repo", "/root/.axon_site/_ro/trn_rl_repo"):
    if os.path.isdir(_p) and _p not in sys.path:
        sys.path.insert(0, _p)

from contextlib import ExitStack

import ml_dtypes

import concourse.bass as bass
import concourse.tile as tile
from concourse import bacc, mybir
from concourse.bass_utils import run_bass_kernel_spmd

H = 256
N_LEN = 256
T_LEN = 1024
BATCH = 64
NCORES = 8
B_LOC = BATCH // NCORES  # 8
P = 128
FP32 = mybir.dt.float32
BF16 = mybir.dt.bfloat16
AF = mybir.ActivationFunctionType
ALU = mybir.AluOpType
BF16_NP = ml_dtypes.bfloat16


def build_program():
    nc = bacc.Bacc(
        "TRN2",
        target_bir_lowering=False,
        debug=False,
        enable_asserts=True,
        num_devices=NCORES,
    )

    # Host-prepared layouts (see make_in_maps):
    #   encT[b, hp, th, hc, t'] = enc[th*512+t', b, hc*128+hp]   bf16
    #   hidT[b, hp, hc, n] = hid[n, b, hc*128+hp]                bf16
    enc_ap = nc.dram_tensor(
        "encT", [B_LOC, P, 2, 2, 512], BF16, kind="ExternalInput"
    ).ap()
    hid_ap = nc.dram_tensor("hidT", [B_LOC, P, 2, N_LEN], BF16, kind="ExternalInput").ap()
    # wpack[:, hc, 0:128]=u2bc, [:, hc, 128]=u1 col hc,
    # [:, 0, 130:132]=fp32 bits of c (bitcast on device)
    wpack_ap = nc.dram_tensor("wpack", [P, 2, 132], BF16, kind="ExternalInput").ap()
    out_ap = nc.dram_tensor(
        "out", [B_LOC, 2, P, T_LEN], BF16, kind="ExternalOutput"
    ).ap()

    out_r = out_ap.rearrange("b nc p t -> b p nc t")  # (8, 128, 2, 1024)

    with tile.TileContext(nc) as tc, ExitStack() as ctx:
        singles = ctx.enter_context(tc.tile_pool(name="singles", bufs=1))
        ps_pre = ctx.enter_context(tc.tile_pool(name="ps_pre", bufs=3, space="PSUM"))
        ps_shc = ctx.enter_context(tc.tile_pool(name="ps_shc", bufs=2, space="PSUM"))
        enc_pool = ctx.enter_context(tc.tile_pool(name="enc", bufs=8))
        hid_pool = ctx.enter_context(tc.tile_pool(name="hid", bufs=8))
        stats = ctx.enter_context(tc.tile_pool(name="stats", bufs=8))
        et_pool = ctx.enter_context(tc.tile_pool(name="et", bufs=2))
        xt_pool = ctx.enter_context(tc.tile_pool(name="xt", bufs=3))
        ot_pool = ctx.enter_context(tc.tile_pool(name="ot", bufs=3))

        # ---- b0's enc halves + hid0 lead the sync ring; wpack rides the
        #      otherwise-idle ACT ring in parallel ----
        enc_sbs, hid_sbs = [], []
        enc0a = enc_pool.tile([P, 2, 512], BF16, tag="enc0a")
        nc.sync.dma_start(enc0a[:], enc_ap[0][:, 0])
        wpack = singles.tile([P, 2, 132], BF16)
        nc.sync.dma_start(wpack[:], wpack_ap)
        u2bc_sb = wpack[:, :, 0:P]
        u1sb = wpack[:, :, P : P + 1]  # (128, 2, 1)
        c_col = wpack[:, 0, 130:132].bitcast(FP32)  # (128, 1) fp32

        for b in range(B_LOC):
            hid_sb = hid_pool.tile([P, 2, N_LEN], BF16)
            if b == 0:
                # b0: enc in two SEPARATE tiles (the dep tracker is
                # whole-tile; split DMAs into one tile create false
                # cross-half dependencies)
                nc.sync.dma_start(hid_sb[:], hid_ap[b])
                enc0b = enc_pool.tile([P, 2, 512], BF16, tag="enc0b")
                nc.sync.dma_start(enc0b[:], enc_ap[b][:, 1])
                enc_sb = (enc0a, enc0b)
            else:
                enc_sb = enc_pool.tile([P, 2, 2, 512], BF16)
                nc.sync.dma_start(enc_sb[:], enc_ap[b])
                nc.sync.dma_start(hid_sb[:], hid_ap[b])
            enc_sbs.append(enc_sb)
            hid_sbs.append(hid_sb)

        # warm the ACT spline tables off the critical path (tanh and exp
        # live in the same table set; one activation loads it)
        warm_in = singles.tile([1, P], BF16)
        nc.vector.memset(warm_in[:], 1.0)
        warm = singles.tile([1, P], FP32)
        nc.scalar.activation(out=warm[:], in_=warm_in[:], func=AF.Tanh)

        # ---- main pipeline ----
        SKEW = 1
        xts, sums_t = {}, {}

        def normalize_and_store(b):
            rsums = stats.tile([P, 2], FP32, tag="rsums")
            nc.vector.reciprocal(rsums[:], sums_t[b][:])
            o_t = ot_pool.tile([P, 2, T_LEN], BF16)
            for j in range(2):
                nc.vector.tensor_scalar_mul(
                    o_t[:, j, :], xts[(b, j)][:], rsums[:, j : j + 1]
                )
            nc.sync.dma_start(out_r[b], o_t[:])

        for b in range(B_LOC):
            enc_sb, hid_sb = enc_sbs[b], hid_sbs[b]

            # shc[:, j] = hid[n, :] . u1 (PE columns); +c on the copy out
            shc_ps = ps_shc.tile([P, 2], FP32)
            for j in range(2):
                for hc in range(2):
                    nc.tensor.matmul(
                        out=shc_ps[:, j : j + 1],
                        lhsT=hid_sb[:, hc, j * P : (j + 1) * P],
                        rhs=u1sb[:, hc, :],
                        start=(hc == 0),
                        stop=(hc == 1),
                    )
            shc = stats.tile([P, 2], FP32, tag="shc")
            nc.vector.tensor_scalar_add(shc[:], shc_ps[:], c_col)

            # pre[n, t] = se[t] for every n-partition (PE broadcast-dot)
            pre_ps = ps_pre.tile([P, T_LEN], FP32)
            for th in range(2):
                rhs_th = enc_sb[th] if b == 0 else enc_sb[:, th]
                for hc in range(2):
                    nc.tensor.matmul(
                        out=pre_ps[:, th * 512 : (th + 1) * 512],
                        lhsT=u2bc_sb[:, hc, :],
                        rhs=rhs_th[:, hc, :],
                        start=(hc == 0),
                        stop=(hc == 1),
                    )

            # transcendentals: 2 bias'd tanh + 1 merged exp per b
            sums = stats.tile([P, 2], FP32, tag="sums")
            sums_t[b] = sums
            last = b == B_LOC - 1
            e2 = et_pool.tile([P, 2, T_LEN], FP32)
            for j in range(2):
                if b == 0 and j == 0:
                    # split so ScalarE starts on enc0's first half early
                    for th in range(2):
                        nc.scalar.activation(
                            out=e2[:, j, th * 512 : (th + 1) * 512],
                            in_=pre_ps[:, th * 512 : (th + 1) * 512],
                            func=AF.Tanh,
                            bias=shc[:, j : j + 1],
                            scale=1.0,
                        )
                else:
                    nc.scalar.activation(
                        out=e2[:, j, :],
                        in_=pre_ps[:],
                        func=AF.Tanh,
                        bias=shc[:, j : j + 1],
                        scale=1.0,
                    )
            x2 = xt_pool.tile([P, 2, T_LEN], BF16)
            if last:
                # tail: per-j exps with accum so normalize starts immediately
                for j in range(2):
                    xl = xt_pool.tile([P, T_LEN], BF16, tag=f"xl{j}")
                    nc.scalar.activation(
                        out=xl[:],
                        in_=e2[:, j, :],
                        func=AF.Exp,
                        accum_out=sums[:, j : j + 1],
                    )
                    xts[(b, j)] = xl
            else:
                nc.scalar.activation(out=x2[:], in_=e2[:], func=AF.Exp)
                nc.vector.tensor_reduce(
                    out=sums[:], in_=x2[:], axis=mybir.AxisListType.X, op=ALU.add
                )
                xts[(b, 0)] = x2[:, 0, :]
                xts[(b, 1)] = x2[:, 1, :]

            if b >= SKEW:
                normalize_and_store(b - SKEW)

        # tail: per-j scale+store for the final batch; stores ride the
        # now-idle ACT ring
        bl = B_LOC - 1
        for j in range(2):
            rs = stats.tile([P, 1], FP32, tag=f"rs{j}")
            nc.vector.reciprocal(rs[:], sums_t[bl][:, j : j + 1])
            o_j = ot_pool.tile([P, T_LEN], BF16, tag=f"oj{j}")
            nc.vector.tensor_scalar_mul(o_j[:], xts[(bl, j)][:], rs[:])
            nc.scalar.dma_start(out_r[bl][:, j, :], o_j[:])

    nc.compile()
    return nc


_CACHE = {}


def get_program():
    if "nc" not in _CACHE:
        _CACHE["nc"] = build_program()
    return _CACHE["nc"]


def make_in_maps(hidden, encoder_outputs, W, b, v):
    # encT[b, hp, th, hc, t'] = enc[th*512+t', b, hc*128+hp]
    encT = np.asarray(encoder_outputs, dtype=np.float32).reshape(2, 512, BATCH, 2, P)
    encT = encT.transpose(2, 4, 0, 3, 1).astype(BF16_NP)  # (64, 128, 2, 2, 512)
    hidT = np.asarray(hidden, dtype=np.float32).reshape(N_LEN, BATCH, 2, P)
    hidT = hidT.transpose(1, 3, 2, 0).astype(BF16_NP)  # (64, 128, 2, 256)

    # replicated small weights, in the reference's own u1/u2/c decomposition
    W32 = np.asarray(W, dtype=np.float32)
    v32 = np.asarray(v, dtype=np.float32)
    b32 = np.asarray(b, dtype=np.float32)
    u1 = W32[:, :H].T @ v32  # (256,)
    u2 = W32[:, H:].T @ v32  # (256,)
    c = float(b32 @ v32)
    # wpack[:, hc, 0:128]=u2bc, [:, hc, 128]=u1 col hc, [:, 0, 130:132]=c bits
    wpack = np.zeros((P, 2, 132), dtype=np.float32)
    wpack[:, :, 0:P] = u2.reshape(2, P).T[:, :, None]
    wpack[:, :, P] = u1.reshape(2, P).T
    wpack = wpack.astype(BF16_NP)
    c_u16 = np.frombuffer(np.float32(c).tobytes(), dtype=np.uint16)
    wp_u16 = wpack.view(np.uint16)
    wp_u16[:, 0, 130] = c_u16[0]
    wp_u16[:, 0, 131] = c_u16[1]

    in_maps = []
    for i in range(NCORES):
        sl = slice(i * B_LOC, (i + 1) * B_LOC)
        in_maps.append(
            {
                "encT": np.ascontiguousarray(encT[sl]),
                "hidT": np.ascontiguousarray(hidT[sl]),
                "wpack": wpack,
            }
        )
    return in_maps


def kernel(hidden, encoder_outputs, W, b, v, _trace=False, _trace_kwargs=None):
    nc = get_program()
    in_maps = make_in_maps(hidden, encoder_outputs, W, b, v)
    res = run_bass_kernel_spmd(
        nc,
        in_maps,
        core_ids=list(range(NCORES)),
        trace=_trace,
        **(_trace_kwargs or {}),
    )
    parts = []
    for i in range(NCORES):
        o = np.asarray(res.results[i]["out"])  # (8, 2, 128, 1024) bf16
        parts.append(o.reshape(B_LOC, N_LEN, T_LEN).astype(np.float32))
    out = np.concatenate(parts, axis=0)
    if _trace:
        return out, res
    return out
